# revision 10
# baseline (speedup 1.0000x reference)
"""DARNN Trainium2 Bass kernel, v2 — instruction-count-minimized design.

Data parallel: B=256 -> 8 cores x BL=32. Weights replicated.

Per-core design notes (b = device batch slot, 0..31):
- Encoder attention: tanh-arg tensor th[(b_lo,u)-part 128, (c, B4, n) free
  2048] where batch b = b_lo*16 + c*4 + B4.  E contracted on PE with a
  v_e-masked (128,2) stationary into ONE psum tile: chunk c -> rows
  {32c, 32c+1} (tile_position (0,32c)).  Softmax along free (reduce-X),
  alpha transposed to (n-part, b) with one XBAR dma_start_transpose, then
  one DVE mul against xT produces the gates lhsT directly.
- Gates computed TRANSPOSED: out (b-part 32, 1024 gates free) via 6
  matmuls (lhsT = x~T / hT[k]), bias added with one DVE op (replicated
  bias tile).  LSTM pointwise in 5 fused scalar_tensor_tensor/ACT ops
  using doubled states H=2h, C=2c (sigmoid(x) = 0.5+0.5tanh(x/2); all
  weight consumers of H/C pre-scaled by 0.5 on host).
- State transposes (b,256)->(128,2,32) are single XBAR DMAs (bf16).
  Encoder h lands directly into XeT[:, :, :, t] (layout (m,2,b,t)).
- Decoder mirrors this: sd^T via 4 matmuls + XBAR; l contracted with
  v_d-folded (128,1) stationaries into psum rows {0,32,64,96}; softmax
  over t in-free; y_tilde assembled on sparse rows and gathered to a
  (2,32) row-tile ([y;1]) that is the gates lhsT (bias folded in rhs).
"""

import sys

for _p in ("/opt/trn_rl_repo", "/root/.axon_site/_ro/trn_rl_repo"):
    if _p not in sys.path:
        sys.path.insert(0, _p)

import numpy as np

B, T, N, M, P, YD = 256, 64, 128, 256, 256, 1
NCORES = 8
BL = B // NCORES
U = T  # 64


def _f32(x):
    return np.ascontiguousarray(x, dtype=np.float32)


def _bf(x):
    import ml_dtypes

    return np.ascontiguousarray(np.asarray(x, np.float32).astype(ml_dtypes.bfloat16))


def _prep_weights(inputs):
    """Host-side weight re-layout + folding (weights only)."""
    WU_e = _f32(inputs["WU_e"])  # (64, 576)
    v_e = _f32(inputs["v_e"])[0]  # (64,)
    WU_d = _f32(inputs["WU_d"])  # (256, 768)
    v_d = _f32(inputs["v_d"])[0]  # (256,)
    wb = _f32(inputs["wb_tilde"])[0]  # (257,)
    Wih_e = _f32(inputs["Wih_e"])  # (1024, 128)
    Whh_e = _f32(inputs["Whh_e"])  # (1024, 256)
    be = _f32(inputs["bih_e"]) + _f32(inputs["bhh_e"])  # (1024,)
    Wih_d = _f32(inputs["Wih_d"])  # (1024, 1)
    Whh_d = _f32(inputs["Whh_d"])  # (1024, 256)
    bd = _f32(inputs["bih_d"]) + _f32(inputs["bhh_d"])  # (1024,)
    Wb_W = _f32(inputs["Wb_W"])  # (256, 512)
    Wb_b = _f32(inputs["Wb_b"])  # (256,)
    vb_W = _f32(inputs["vb_W"])  # (1, 256)
    vb_b = _f32(inputs["vb_b"])  # (1,)

    Wh_e = WU_e[:, : 2 * M]  # (64, 512) cols [h(256); c(256)]
    Wx_e = WU_e[:, 2 * M :]  # (64, 64)
    Wh_d = WU_d[:, : 2 * P]  # (256, 512)
    Wx_d = WU_d[:, 2 * P :]  # (256, 256)

    # sigmoid-arg fold: i,f,o rows scaled 0.5 (tanh(x/2) trick); g rows 1.0
    gs = np.ones((4 * M,), np.float32)
    gs[0 : 2 * M] = 0.5
    gs[3 * M :] = 0.5

    w = {}

    # --- encoder attention ---
    w["vrep32"] = _bf(np.broadcast_to(v_e, (BL, 64)))
    # ewh[r, k, u] = 0.5 * Wh_e[u, 128k + r]   (0.5 = H/C-fold)
    w["ewh"] = _bf(
        np.stack([0.5 * Wh_e[:, 128 * k : 128 * (k + 1)].T for k in range(4)], axis=1)
    )  # (128, 4, 64)

    # --- encoder LSTM ---
    Wih_s = Wih_e * gs[:, None]
    Whh_s = Whh_e * gs[:, None] * 0.5  # H-fold
    # wihT[n, ch, col] = Wih_s[512ch + col, n]
    w["wihT"] = _bf(
        np.stack([Wih_s[512 * c : 512 * (c + 1), :].T for c in range(2)], axis=1)
    )  # (128, 2, 512)
    w["whhT"] = _bf(
        np.stack(
            [
                np.stack(
                    [Whh_s[512 * c : 512 * (c + 1), 128 * k : 128 * (k + 1)].T
                     for c in range(2)],
                    axis=1,
                )
                for k in range(2)
            ],
            axis=1,
        )
    )  # (128, 2, 2, 512)
    w["biasrep"] = _f32(np.broadcast_to(be * gs, (BL, 1024)))

    # --- decoder ---
    # whdT[r, k, m'] = 0.5 * Wh_d[m', 128k + r]
    w["whdT"] = _bf(
        np.stack([0.5 * Wh_d[:, 128 * k : 128 * (k + 1)].T for k in range(4)], axis=1)
    )  # (128, 4, 256)
    # wxd[r, k, mt, c] = 0.5 * Wx_d[128mt + c, 128k + r]
    w["wxd"] = _bf(
        np.stack(
            [
                np.stack(
                    [0.5 * Wx_d[128 * mt : 128 * (mt + 1), 128 * k : 128 * (k + 1)].T
                     for mt in range(2)],
                    axis=1,
                )
                for k in range(2)
            ],
            axis=1,
        )
    )  # (128, 2, 2, 128)
    w["vdk"] = _bf(np.stack([v_d[0:128], v_d[128:256]], axis=1))  # (128, 2)
    wbm = wb[1:]  # (256,)
    w["wbmk"] = _bf(0.5 * np.stack([wbm[0:128], wbm[128:256]], axis=1))
    w_eff = Wb_W.T @ vb_W.T  # (512, 1)
    w2 = w_eff[256:, 0]
    w["w2k"] = _bf(0.5 * np.stack([w2[0:128], w2[128:256]], axis=1))
    w["w1f"] = _bf(0.5 * np.stack([w_eff[0:128, 0], w_eff[128:256, 0]], axis=1))
    Wih_ds = Wih_d[:, 0] * gs
    Whh_ds = Whh_d * gs[:, None] * 0.5
    w["wyrep"] = _f32(np.broadcast_to(Wih_ds, (BL, 1024)))
    w["biasrepd"] = _f32(np.broadcast_to(bd * gs, (BL, 1024)))
    w["whhdT"] = _bf(
        np.stack(
            [
                np.stack(
                    [Whh_ds[512 * c : 512 * (c + 1), 128 * k : 128 * (k + 1)].T
                     for c in range(2)],
                    axis=1,
                )
                for k in range(2)
            ],
            axis=1,
        )
    )  # (128, 2, 2, 512)

    scalars = {
        "wb0": float(wb[0]),
        "c_eff": float((Wb_b @ vb_W.T + vb_b)[0]),
        "Wx_e": Wx_e,
    }
    return w, scalars


def _prep_core_inputs(Xc, Yc, Wx_e, wb0):
    """Per-core input tensors in device layouts.

    Xc: (BL, T, N) float32, Yc: (BL, T) float32.
    """
    out = {}
    out["x32"] = _bf(Xc)  # (32, 64, 128)
    # A5T[b, n, u] = sum_t X[b,t,n] * Wx_e[u,t]
    out["A5T"] = _bf(np.einsum("btn,ut->bnu", Xc, Wx_e))  # (32, 128, 64)
    # ysc[row 32c, b_loc, t] = wb0 * Y[8c + b_loc, t]   for t < T-1
    ysc = np.zeros((128, 8, T - 1), np.float32)
    for ch in range(4):
        ysc[32 * ch, :, :] = wb0 * Yc[8 * ch : 8 * (ch + 1), : T - 1]
    out["ysc"] = _f32(ysc)
    return out


def _build(w_shapes, scalars):
    import concourse.bass as bass
    import concourse.bacc as bacc
    import concourse.tile as tile
    from concourse import mybir
    import contextlib

    fp32 = mybir.dt.float32
    bf16 = mybir.dt.bfloat16
    AF = mybir.ActivationFunctionType
    OP = mybir.AluOpType

    c_eff = scalars["c_eff"]

    nc = bacc.Bacc()
    dram = {
        name: nc.dram_tensor(name, list(shape), dt, kind="ExternalInput")
        for name, (shape, dt) in w_shapes.items()
    }
    out_d = nc.dram_tensor("out", [BL, YD], fp32, kind="ExternalOutput")

    with tile.TileContext(nc) as tc:
        ctx = contextlib.ExitStack()
        with ctx:
            sing = ctx.enter_context(tc.tile_pool(name="sing", bufs=1))

            sb = {}
            for name, t_ in dram.items():
                if name in ("x32", "A5T"):
                    continue
                til = sing.tile(list(t_.shape), t_.dtype, name=f"w_{name}",
                                tag=f"w_{name}")
                nc.gpsimd.dma_start(out=til, in_=t_.ap())
                sb[name] = til

            # persistent state / shared tiles
            hcH = sing.tile([128, T, 4, BL], bf16, name="hcH", tag="hcH")
            hcs_prev = [None]
            Xe_bt = sing.tile([128, 2, BL, T], bf16, name="Xe_bt", tag="Xe_bt")

            ps_s = ctx.enter_context(tc.tile_pool(name="ps_s", bufs=1, space="PSUM"))
            ps_g = ctx.enter_context(tc.tile_pool(name="ps_g", bufs=1, space="PSUM"))

            enc = ctx.enter_context(contextlib.ExitStack())
            ep = enc.enter_context(tc.tile_pool(name="ep", bufs=4))
            epb = enc.enter_context(tc.tile_pool(name="epb", bufs=1))
            encin = enc.enter_context(tc.tile_pool(name="encin", bufs=1))
            for name in ("x32", "A5T"):
                t_ = dram[name]
                til = encin.tile(list(t_.shape), t_.dtype, name=f"w_{name}",
                                 tag=f"w_{name}")
                nc.gpsimd.dma_start(out=til, in_=t_.ap())
                sb[name] = til
            ep3 = enc.enter_context(tc.tile_pool(name="ep3", bufs=6))

            for t in range(T):
                # ---- attention ----
                if t > 0:
                    s_ps = ps_s.tile([BL, 64], fp32, tag="s", name="s_ps")
                    for k in range(4):
                        nc.tensor.matmul(
                            s_ps,
                            lhsT=hcH[:, t - 1, k, :],
                            rhs=sb["ewh"][:, k, :],
                            start=(k == 0),
                            stop=(k == 3),
                        )
                    th = epb.tile([BL, 128, 64], bf16, tag="th")
                    nc.vector.tensor_add(
                        th,
                        sb["A5T"],
                        s_ps[:, None, :].broadcast_to([BL, 128, 64]),
                    )
                    tha = th
                else:
                    tha = sb["A5T"]
                tht = epb.tile([BL, 128, 64], bf16, tag="tht")
                nc.scalar.activation(
                    out=tht.rearrange("p n u -> p (n u)"),
                    in_=tha.rearrange("p n u -> p (n u)"),
                    func=AF.Tanh,
                )
                vth = epb.tile([BL, 128, 64], bf16, tag="vth")
                nc.vector.tensor_mul(
                    vth, tht, sb["vrep32"][:, None, :].broadcast_to([BL, 128, 64])
                )
                E_sb = ep.tile([BL, 128], fp32, tag="E_sb")
                nc.vector.tensor_reduce(
                    out=E_sb, in_=vth, axis=mybir.AxisListType.X, op=OP.add
                )
                expE = ep.tile([BL, 128], fp32, tag="expE")
                den = ep.tile([BL, 1], fp32, tag="den")
                nc.scalar.activation(
                    out=expE, in_=E_sb, func=AF.Exp, accum_out=den
                )
                inv = ep.tile([BL, 1], fp32, tag="inv")
                nc.vector.reciprocal(out=inv, in_=den)
                xst = ep.tile([BL, 128], bf16, tag="xst")
                nc.vector.scalar_tensor_tensor(
                    out=xst, in0=expE, scalar=inv, in1=sb["x32"][:, t, :],
                    op0=OP.mult, op1=OP.mult,
                )
                xaT = ep.tile([128, BL], bf16, tag="xaT")
                nc.sync.dma_start_transpose(out=xaT, in_=xst)
                # ---- gates ----
                g_ps = ps_g.tile([BL, 2, 512], fp32, tag="g", name="g_ps")
                for ch in range(2):
                    nc.tensor.matmul(
                        g_ps[:, ch, :],
                        lhsT=xaT,
                        rhs=sb["wihT"][:, ch, :],
                        start=True,
                        stop=(t == 0),
                    )
                if t > 0:
                    for k in range(2):
                        for ch in range(2):
                            nc.tensor.matmul(
                                g_ps[:, ch, :],
                                lhsT=hcH[:, t - 1, k, :],
                                rhs=sb["whhT"][:, k, ch, :],
                                start=False,
                                stop=(k == 1),
                            )
                gsb = ep.tile([BL, 1024], fp32, tag="gsb")
                nc.vector.tensor_add(
                    gsb, g_ps.rearrange("p c f -> p (c f)"), sb["biasrep"]
                )
                tg = ep.tile([BL, 1024], fp32, tag="tg")
                nc.scalar.activation(out=tg, in_=gsb, func=AF.Tanh)
                # ---- pointwise (H=2h, C=2c) ----
                hcs = ep3.tile([BL, 512], bf16, tag="hcs")
                Cn = hcs[:, 256:512]
                if t > 0:
                    wt = ep3.tile([BL, 256], fp32, tag="wt")
                    nc.vector.scalar_tensor_tensor(
                        out=wt, in0=tg[:, 0:256], scalar=1.0, in1=tg[:, 512:768],
                        op0=OP.add, op1=OP.mult,
                    )
                    ut = ep3.tile([BL, 256], fp32, tag="ut")
                    nc.vector.scalar_tensor_tensor(
                        out=ut, in0=tg[:, 256:512], scalar=1.0,
                        in1=hcs_prev[0][:, 256:512],
                        op0=OP.add, op1=OP.mult,
                    )
                    nc.vector.scalar_tensor_tensor(
                        out=Cn, in0=ut, scalar=0.5, in1=wt, op0=OP.mult, op1=OP.add
                    )
                else:
                    nc.vector.scalar_tensor_tensor(
                        out=Cn, in0=tg[:, 0:256], scalar=1.0, in1=tg[:, 512:768],
                        op0=OP.add, op1=OP.mult,
                    )
                tc_ = ep3.tile([BL, 256], fp32, tag="tc")
                nc.scalar.activation(out=tc_, in_=Cn, func=AF.Tanh, scale=0.5)
                nc.vector.scalar_tensor_tensor(
                    out=hcs[:, 0:256], in0=tg[:, 768:1024], scalar=1.0, in1=tc_,
                    op0=OP.add, op1=OP.mult,
                )
                nc.sync.dma_start_transpose(out=hcH[:, t, :, :], in_=hcs)
                hcs_prev[0] = hcs

            enc.close()

            # ---------- decoder precompute: AX, q, q2 ----------
            pre = ctx.enter_context(contextlib.ExitStack())
            pp = pre.enter_context(tc.tile_pool(name="pp", bufs=2))
            ps_p = pre.enter_context(tc.tile_pool(name="ps_p", bufs=1, space="PSUM"))
            nc.vector.tensor_copy(out=Xe_bt,
                                  in_=hcH[:, :, 0:2, :].rearrange(
                                      "p t k b -> p k b t"))
            AX = sing.tile([128, 2, BL, T], bf16, name="AX", tag="AX")
            qt = sing.tile([128, 8, T], fp32, name="qt", tag="qt")
            q2t = sing.tile([128, 8, T], fp32, name="q2t", tag="q2t")
            for mt in range(2):
                for pair in range(2):
                    chs = (2 * pair, 2 * pair + 1)
                    axps = {ch: ps_p.tile([128, 512], fp32, tag=f"axp{ch % 2}",
                                          name="axp") for ch in chs}
                    for k in range(2):
                        for ch in chs:
                            nc.tensor.matmul(
                                axps[ch],
                                lhsT=sb["wxd"][:, k, mt, :],
                                rhs=Xe_bt[:, k].rearrange("p b t -> p (b t)")[
                                    :, 512 * ch : 512 * (ch + 1)
                                ],
                                start=(k == 0),
                                stop=(k == 1),
                            )
                    for ch in chs:
                        nc.vector.tensor_copy(
                            out=AX[:, mt].rearrange("p b t -> p (b t)")[
                                :, 512 * ch : 512 * (ch + 1)
                            ],
                            in_=axps[ch],
                        )
            for src, dst in ((sb["wbmk"], qt), (sb["w2k"], q2t)):
                qp = ps_p.tile([128, 512], fp32, tag="qp", name="qp")
                for ch in range(4):
                    for k in range(2):
                        nc.tensor.matmul(
                            qp[32 * ch : 32 * ch + 1, :],
                            lhsT=src[:, k : k + 1],
                            rhs=Xe_bt[:, k].rearrange("p b t -> p (b t)")[
                                :, 512 * ch : 512 * (ch + 1)
                            ],
                            start=(k == 0),
                            stop=(k == 1),
                            tile_position=(0, 32 * ch),
                        )
                nc.vector.tensor_copy(
                    out=dst.rearrange("p b t -> p (b t)"), in_=qp
                )
            pre.close()

            # ---------- decoder ----------
            dec = ctx.enter_context(contextlib.ExitStack())
            dp = dec.enter_context(tc.tile_pool(name="dp", bufs=2))
            dp3 = dec.enter_context(tc.tile_pool(name="dp3", bufs=6))
            ps_l = ctx.enter_context(tc.tile_pool(name="ps_l", bufs=1, space="PSUM"))

            hcdT = [sing.tile([128, 4, BL], bf16, name=f"hcdT{i}", tag=f"hcdT{i}")
                    for i in range(2)]
            hcds_prev = [None]
            y_col = sing.tile([BL, 1], fp32, name="y_col", tag="y_col")

            expl_f = inv_f = None
            for t in range(T - 1):
                if t > 0:
                    sd_ps = ps_s.tile([BL, 256], fp32, tag="sd", name="sd_ps")
                    for k in range(4):
                        nc.tensor.matmul(
                            sd_ps,
                            lhsT=hcdT[(t - 1) % 2][:, k, :],
                            rhs=sb["whdT"][:, k, :],
                            start=(k == 0),
                            stop=(k == 3),
                        )
                    sdbf = dp.tile([BL, 256], bf16, tag="sdbf")
                    nc.vector.tensor_copy(out=sdbf, in_=sd_ps)
                    sdT = dp.tile([128, 2, BL], bf16, tag="sdT")
                    nc.sync.dma_start_transpose(out=sdT, in_=sdbf)
                    thd = dp.tile([128, 2, BL, T], bf16, tag="thd")
                    nc.vector.tensor_add(
                        thd, AX, sdT[:, :, :, None].broadcast_to([128, 2, BL, T])
                    )
                    tsrc = thd
                else:
                    tsrc = AX
                thdt = dp.tile([128, 2, BL, T], bf16, tag="thdt")
                nc.scalar.activation(
                    out=thdt.rearrange("p k b t -> p (k b t)"),
                    in_=tsrc.rearrange("p k b t -> p (k b t)"),
                    func=AF.Tanh,
                )
                l_ps = ps_l.tile([128, 512], fp32, tag="l", name="l_ps")
                for k in range(2):
                    for ch in range(4):
                        nc.tensor.matmul(
                            l_ps[32 * ch : 32 * ch + 1, :],
                            lhsT=sb["vdk"][:, k : k + 1],
                            rhs=thdt[:, k].rearrange("p b t -> p (b t)")[
                                :, 512 * ch : 512 * (ch + 1)
                            ],
                            start=(k == 0),
                            stop=(k == 1),
                            tile_position=(0, 32 * ch),
                        )
                expl = dp.tile([128, 8, T], fp32, tag="expl")
                nc.scalar.activation(
                    out=expl.rearrange("p b t -> p (b t)"), in_=l_ps, func=AF.Exp
                )
                den = dp.tile([128, 8], fp32, tag="dend")
                nc.vector.tensor_reduce(
                    out=den, in_=expl, axis=mybir.AxisListType.X, op=OP.add
                )
                inv = dp.tile([128, 8], fp32, tag="invd")
                nc.vector.reciprocal(out=inv, in_=den)
                eq = dp.tile([128, 8, T], fp32, tag="eq")
                nc.vector.tensor_mul(eq, expl, qt)
                num = dp.tile([128, 8], fp32, tag="num")
                nc.vector.tensor_reduce(
                    out=num, in_=eq, axis=mybir.AxisListType.X, op=OP.add
                )
                nd = dp.tile([128, 8], fp32, tag="nd")
                nc.vector.tensor_mul(nd, num, inv)
                yt = dp.tile([128, 8], fp32, tag="yt")
                nc.vector.tensor_add(yt, nd, sb["ysc"][:, :, t])
                nc.sync.dma_start(
                    out=y_col,
                    in_=yt.rearrange("(c r) f -> c r f", c=4)[:, 0:1, :],
                )
                expl_f, inv_f = expl, inv
                # gates
                ytmp = dp.tile([BL, 1024], fp32, tag="ytmp")
                nc.vector.scalar_tensor_tensor(
                    out=ytmp,
                    in0=sb["wyrep"],
                    scalar=y_col,
                    in1=sb["biasrepd"],
                    op0=OP.mult,
                    op1=OP.add,
                )
                if t > 0:
                    g_ps = ps_g.tile([BL, 2, 512], fp32, tag="g", name="g_ps")
                    for k in range(2):
                        for ch in range(2):
                            nc.tensor.matmul(
                                g_ps[:, ch, :],
                                lhsT=hcdT[(t - 1) % 2][:, k, :],
                                rhs=sb["whhdT"][:, k, ch, :],
                                start=(k == 0),
                                stop=(k == 1),
                            )
                    gsb = dp.tile([BL, 1024], fp32, tag="gsb2")
                    nc.vector.tensor_add(
                        gsb, ytmp, g_ps.rearrange("p c f -> p (c f)")
                    )
                    tgsrc = gsb
                else:
                    tgsrc = ytmp
                tg = dp.tile([BL, 1024], fp32, tag="tg")
                nc.scalar.activation(out=tg, in_=tgsrc, func=AF.Tanh)
                hcs = dp3.tile([BL, 512], bf16, tag="hcs")
                Cn = hcs[:, 256:512]
                if t > 0:
                    wt = dp3.tile([BL, 256], fp32, tag="wt")
                    nc.vector.scalar_tensor_tensor(
                        out=wt, in0=tg[:, 0:256], scalar=1.0, in1=tg[:, 512:768],
                        op0=OP.add, op1=OP.mult,
                    )
                    ut = dp3.tile([BL, 256], fp32, tag="ut")
                    nc.vector.scalar_tensor_tensor(
                        out=ut, in0=tg[:, 256:512], scalar=1.0,
                        in1=hcds_prev[0][:, 256:512],
                        op0=OP.add, op1=OP.mult,
                    )
                    nc.vector.scalar_tensor_tensor(
                        out=Cn, in0=ut, scalar=0.5, in1=wt, op0=OP.mult, op1=OP.add
                    )
                else:
                    nc.vector.scalar_tensor_tensor(
                        out=Cn, in0=tg[:, 0:256], scalar=1.0, in1=tg[:, 512:768],
                        op0=OP.add, op1=OP.mult,
                    )
                tc_ = dp3.tile([BL, 256], fp32, tag="tc")
                nc.scalar.activation(out=tc_, in_=Cn, func=AF.Tanh, scale=0.5)
                nc.vector.scalar_tensor_tensor(
                    out=hcs[:, 0:256], in0=tg[:, 768:1024], scalar=1.0, in1=tc_,
                    op0=OP.add, op1=OP.mult,
                )
                nc.sync.dma_start_transpose(out=hcdT[t % 2], in_=hcs)
                hcds_prev[0] = hcs

            # ---------- output ----------
            tlast = (T - 2) % 2
            eq2 = dp.tile([128, 8, T], fp32, tag="eq")
            nc.vector.tensor_mul(eq2, expl_f, q2t)
            num2 = dp.tile([128, 8], fp32, tag="num")
            nc.vector.tensor_reduce(
                out=num2, in_=eq2, axis=mybir.AxisListType.X, op=OP.add
            )
            yctx = dp.tile([128, 8], fp32, tag="yctx")
            nc.vector.tensor_mul(yctx, num2, inv_f)
            ycrow = dp.tile([1, BL], fp32, tag="ycrow")
            nc.sync.dma_start(
                out=ycrow.rearrange("o (c f) -> o c f", c=4),
                in_=yctx.rearrange("(c r) f -> c r f", c=4)[:, 0, :],
            )
            yh_ps = ps_l.tile([128, 512], fp32, tag="l", name="yh_ps")[0:1, 0:BL]
            for k in range(2):
                nc.tensor.matmul(
                    yh_ps,
                    lhsT=sb["w1f"][:, k : k + 1],
                    rhs=hcdT[tlast][:, k, :],
                    start=(k == 0),
                    stop=(k == 1),
                )
            ysum = dp.tile([1, BL], fp32, tag="ysum")
            nc.vector.tensor_add(ysum, yh_ps, ycrow)
            yfin = dp.tile([1, BL], fp32, tag="yfin")
            nc.vector.tensor_scalar_add(yfin, ysum, c_eff)
            nc.sync.dma_start(out=out_d.ap(), in_=yfin)
            dec.close()
    nc.finalize()
    return nc


_prog_cache = {}


def kernel(**inputs):
    from concourse import mybir
    from concourse.bass_utils import run_bass_kernel_spmd

    w, scalars = _prep_weights(inputs)
    full = _f32(inputs["inputs"])  # (256, 64, 129)

    core_inputs = []
    for c in range(NCORES):
        sh = full[c * BL : (c + 1) * BL]
        ci = _prep_core_inputs(sh[:, :, :N], sh[:, :, N], scalars["Wx_e"],
                               scalars["wb0"])
        ci.update(w)
        core_inputs.append(ci)

    fp32 = mybir.dt.float32
    bf16 = mybir.dt.bfloat16
    dt_map = {2: bf16, 4: fp32}
    w_shapes = {
        name: (arr.shape, dt_map[arr.dtype.itemsize])
        for name, arr in core_inputs[0].items()
    }

    key = ("v2", tuple(sorted((k, tuple(s), str(d)) for k, (s, d) in
                              w_shapes.items())), scalars["c_eff"])
    if key not in _prog_cache:
        _prog_cache[key] = _build(w_shapes, scalars)
    nc = _prog_cache[key]

    import os

    trace = os.environ.get("DARNN_TRACE", "0") != "0"
    res = run_bass_kernel_spmd(
        nc, core_inputs, core_ids=list(range(NCORES)), trace=trace
    )
    global LAST_RESULT
    LAST_RESULT = res
    out = np.concatenate([r["out"] for r in res.results], axis=0)
    return out


LAST_RESULT = None
